# revision 25
# baseline (speedup 1.0000x reference)
"""BatchTopK SAE encoder on 8 Trainium2 NeuronCores (fp8 DoubleRow GEMM).

Strategy
--------
Tensor-parallel over dict_size: core c computes the encoder GEMM for dict
rows [c*4096, (c+1)*4096):

    acts_c^T [4096, 2048] = relu((W_c*64)_fp8 @ (x - b_dec)_fp8^T / 64 + b_enc_c)

as float8e4 (e4m3) matmuls in MatmulPerfMode.DoubleRow — 2 MACs/PE/cycle,
~157 TF/s/core, 2x the f32r/bf16 rate. The fp8 quantization error
(std ~0.038*sigma) is absorbed by the host-side selection: a conservative
threshold screens candidates from the device f16 activations, the
(k*B)-th largest device value defines the cut, and everything within the
error band is recomputed exactly in fp32 from the original inputs so the
selected set matches an exact-fp32 reference.

The kernel returns scatter(top-(k*B) values) as a dense [B, D_DICT] fp32
array, matching the reference semantics (ties broken by lower flat index).
"""

import sys

sys.path.insert(0, "/opt/trn_rl_repo")

import numpy as np

# ---- problem constants (from the spec; asserted at runtime) ----
B = 2048           # batch
D = 2048           # activation dim (contraction)
DD = 32768         # dict size
NCORES = 8
FSH = DD // NCORES # 4096 dict rows per core
KT = D // 128      # 16 contraction tiles
FT = FSH // 128    # 32 f-tiles per core
NB = B // 512      # 4 batch chunks of 512
SW = 64.0          # pow2 weight scale so W*SW ~ N(0,1.28) fits e4m3 well

_STATE = {}


def _build_nc():
    from concourse import bacc
    import concourse.mybir as mybir
    import concourse.tile as tile

    F32 = mybir.dt.float32
    F16 = mybir.dt.float16
    F8 = mybir.dt.float8e4
    RELU = mybir.ActivationFunctionType.Relu
    DROW = mybir.MatmulPerfMode.DoubleRow

    nc = bacc.Bacc("TRN2", target_bir_lowering=False, debug=False, num_devices=NCORES)
    # 2D dram layouts: per-partition rows are contiguous per tile slice, so
    # each tile DMA is 128 descriptors of 1-8KB instead of thousands of
    # 128-512B packets (which starved the PE for ~10us at kernel start)
    xt_d = nc.dram_tensor("xt", [128, NB * KT * 512], F8, kind="ExternalInput").ap()
    wt_d = nc.dram_tensor("wt", [128, FT * KT * 128], F8, kind="ExternalInput").ap()
    be_d = nc.dram_tensor("be", [128, FT], F32, kind="ExternalInput").ap()
    acts_d = nc.dram_tensor("acts", [FSH, B], F16, kind="ExternalOutput").ap()

    FA = 8  # phase-A f-tiles (resident W) — covers PE while x streams in

    with tile.TileContext(nc) as tc:
        with (
            tc.tile_pool(name="xres", bufs=1) as xpool,
            tc.tile_pool(name="wa", bufs=1) as wapool,
            tc.tile_pool(name="wstream", bufs=3) as wpool,
            tc.tile_pool(name="eplg", bufs=8) as opool,
            tc.tile_pool(name="ps", bufs=2, space="PSUM") as pspool,
        ):
            # DMA issue order matters: transfers complete in queue order.
            # Priority: wa0 + the nb0 x pair-tiles (to start the first chain
            # ASAP), then the remaining phase-A W tiles (consumed at 1.7us
            # per chain), and only then the x chunks for nb1-3 (not needed
            # until ~14us+).
            xts = [None] * NB
            was = [None] * FA

            def load_wa(f):
                wa = wapool.tile([128, KT, 128], F8, tag=f"wa{f}")
                nc.sync.dma_start(
                    out=wa, in_=wt_d[:, f * KT * 128 : (f + 1) * KT * 128]
                )
                was[f] = wa

            # f0's weights as 8 kk-pair tiles: the first LDWEIGHTS only waits
            # on 33KB, so the PE starts as soon as the ring delivers anything
            w0p = [
                wapool.tile([128, 2, 128], F8, tag=f"w0p{j}", name=f"w0p{j}")
                for j in range(KT // 2)
            ]

            def load_xnb(nb):
                xnb = xpool.tile([128, KT, 512], F8, tag=f"xt{nb}")
                nc.sync.dma_start(
                    out=xnb, in_=xt_d[:, nb * KT * 512 : (nb + 1) * KT * 512]
                )
                xts[nb] = xnb

            # nb0 split into two kk-pair tiles + two 6-kk rest tiles so the
            # first chain's matmuls start as soon as the first slices land.
            # DMA delivery only begins ~8us in (ring init) at ~300GB/s, so the
            # queue is ordered by first-use time: wa0+nb0 x, then the other
            # phase-A W tiles (one per 1.7us chain), then x for nb1-3
            # (first needed at ~chain 8).
            x0p0 = xpool.tile([128, 2, 512], F8, tag="x0p0")
            x0p1 = xpool.tile([128, 2, 512], F8, tag="x0p1")
            x0r1 = xpool.tile([128, 6, 512], F8, tag="x0r1")
            x0r2 = xpool.tile([128, 6, 512], F8, tag="x0r2")
            be = xpool.tile([128, FT], F32, tag="be")

            # W is the scarce early resource (consumed at 0.26MB/1.73us-chain
            # against ~0.3MB/us total delivery): pull wa1-2 ahead of the x0
            # tail so chains 1..7 never wait on weights; chain 0's later
            # matmuls absorb the x0r wait inside the pipeline instead.
            nc.sync.dma_start(out=w0p[0], in_=wt_d[:, 0 : 2 * 128])
            nc.sync.dma_start(out=x0p0, in_=xt_d[:, 0 : 2 * 512])
            nc.sync.dma_start(out=w0p[1], in_=wt_d[:, 2 * 128 : 4 * 128])
            nc.sync.dma_start(out=x0p1, in_=xt_d[:, 2 * 512 : 4 * 512])
            for j in range(2, KT // 2):
                nc.sync.dma_start(
                    out=w0p[j], in_=wt_d[:, 2 * j * 128 : 2 * (j + 1) * 128]
                )
            nc.sync.dma_start(out=x0r1, in_=xt_d[:, 4 * 512 : 10 * 512])
            load_wa(1)
            nc.sync.dma_start(out=x0r2, in_=xt_d[:, 10 * 512 : 16 * 512])
            load_wa(2)
            nc.sync.dma_start(out=be, in_=be_d)
            for f in range(3, FA):
                load_wa(f)
            # x for nb1-3 is NOT queued here: all 8 cores pulling their full
            # 12.6MB at once saturates chip HBM during the critical first
            # ~15us (the graded time is the slowest core). The x1-3 triggers
            # are interleaved into phase A below, where the sync queue gates
            # them behind early activations — after the W burst has landed.

            chain_no = [0]

            def chain(f, nb, wt):
                # rotate psum tags globally: 4 tags x 2 bufs = 8 banks, so a
                # bank is only reused 8 chains later — the epilogue has ~13us
                # of slack instead of gating the PE after 2 chains
                ps = pspool.tile([128, 512], F32, tag=f"ps{chain_no[0] % 4}")
                chain_no[0] += 1
                for kk in range(0, KT, 2):
                    if nb == 0:
                        if kk == 0:
                            rhs = x0p0
                        elif kk == 2:
                            rhs = x0p1
                        elif kk < 10:
                            rhs = x0r1[:, kk - 4 : kk - 2, :]
                        else:
                            rhs = x0r2[:, kk - 10 : kk - 8, :]
                    else:
                        rhs = xts[nb][:, kk : kk + 2, :]
                    lhsT = w0p[kk // 2] if wt is None else wt[:, kk : kk + 2, :]
                    nc.tensor.matmul(
                        ps,
                        lhsT,
                        rhs,
                        start=(kk == 0),
                        stop=(kk == KT - 2),
                        perf_mode=DROW,
                    )
                ot = opool.tile([128, 512], F16, tag="ot")
                nc.scalar.activation(
                    ot, ps, func=RELU, bias=be[:, f : f + 1], scale=1.0 / SW
                )
                # outputs share the sync ring with inputs ON PURPOSE: ring
                # order gives the input stream absolute priority during the
                # startup window where the PE is delivery-bound. The out-tile
                # pool is 8 deep so the epilogue can lag ~14us behind the PE
                # while early outputs wait for the input stream to drain.
                nc.sync.dma_start(
                    out=acts_d[f * 128 : (f + 1) * 128, nb * 512 : (nb + 1) * 512],
                    in_=ot,
                )

            # phase A: nb-major staircase over the resident f-tiles, keeping
            # the PE busy while the rest of x is still loading. x1-3 loads
            # are interleaved so their triggers (gated behind the preceding
            # chains' epilogues on the sync queue) fire after the W burst.
            for nb in range(NB):
                for f in range(FA):
                    chain(f, nb, None if f == 0 else was[f])
                    ci = nb * FA + f
                    if ci == 1:
                        load_xnb(1)
                    elif ci == 5:
                        load_xnb(2)
                    elif ci == 9:
                        load_xnb(3)

            def rhs_for(nb, kk):
                if nb == 0:
                    if kk == 0:
                        return x0p0
                    if kk == 2:
                        return x0p1
                    if kk < 10:
                        return x0r1[:, kk - 4 : kk - 2, :]
                    return x0r2[:, kk - 10 : kk - 8, :]
                return xts[nb][:, kk : kk + 2, :]

            # phase B: stream the remaining W tiles. kk-major with nb inner:
            # 4 interleaved psum accumulation groups share each weight slice
            # across 4 consecutive matmuls (one stationary load per 4 mms
            # if the lowering dedups identical LDWEIGHTS). The last f-tile
            # keeps the chain-major order so its epilogues pipeline inside
            # the tile instead of all trailing the final matmul.
            for f in range(FA, FT):
                wt = wpool.tile([128, KT, 128], F8, tag="wt")
                nc.sync.dma_start(
                    out=wt, in_=wt_d[:, f * KT * 128 : (f + 1) * KT * 128]
                )
                if f == FT - 1:
                    for nb in range(NB):
                        chain(f, nb, wt)
                    continue
                pss = []
                for nb in range(NB):
                    ps = pspool.tile(
                        [128, 512], F32, tag=f"ps{chain_no[0] % 4}", name=f"psb{nb}"
                    )
                    chain_no[0] += 1
                    pss.append(ps)
                for kk in range(0, KT, 2):
                    for nb in range(NB):
                        nc.tensor.matmul(
                            pss[nb],
                            wt[:, kk : kk + 2, :],
                            rhs_for(nb, kk),
                            start=(kk == 0),
                            stop=(kk == KT - 2),
                            perf_mode=DROW,
                        )
                for nb in range(NB):
                    ot = opool.tile([128, 512], F16, tag="ot")
                    nc.scalar.activation(
                        ot,
                        pss[nb],
                        func=RELU,
                        bias=be[:, f : f + 1],
                        scale=1.0 / SW,
                    )
                    nc.sync.dma_start(
                        out=acts_d[
                            f * 128 : (f + 1) * 128, nb * 512 : (nb + 1) * 512
                        ],
                        in_=ot,
                    )

    nc.compile()
    return nc


def _get_nc():
    if "nc" not in _STATE:
        _STATE["nc"] = _build_nc()
    return _STATE["nc"]


def _fp8(a):
    import ml_dtypes

    return a.astype(ml_dtypes.float8_e4m3)


def _pack_x(xc):
    # xc [B, D] -> fp8 [128, NB*KT*512]: block (nb, kk) holds xc^T[kk*128+p, nb*512+bb]
    return np.ascontiguousarray(
        _fp8(xc).T.reshape(KT, 128, NB, 512).transpose(1, 2, 0, 3).reshape(128, -1)
    )


def _pack_w(Wsh):
    # Wsh [FSH, D] -> fp8 [128, FT*KT*128]: block (f, kk) holds (SW*W)[f*128+m, kk*128+p]
    return np.ascontiguousarray(
        _fp8(Wsh * np.float32(SW))
        .reshape(FT, 128, KT, 128)
        .transpose(3, 0, 2, 1)
        .reshape(128, -1)
    )


def _get_runner():
    """Build the Bass program once and return a cached jitted SPMD callable.

    runner(xt, wt_concat, be_concat) -> actsT [DD, B] (numpy).
    xt is replicated to all 8 cores; wt/be are sharded along axis 0.
    """
    if "runner" in _STATE:
        return _STATE["runner"]

    import jax
    from jax.sharding import Mesh, PartitionSpec
    from jax.experimental.shard_map import shard_map
    from concourse import mybir
    from concourse.bass2jax import (
        _bass_exec_p,
        install_neuronx_cc_hook,
        partition_id_tensor,
    )

    nc = _get_nc()
    install_neuronx_cc_hook()

    pname = nc.partition_id_tensor.name if nc.partition_id_tensor else None
    in_names, out_names, out_avals = [], [], []
    for alloc in nc.m.functions[0].allocations:
        if not isinstance(alloc, mybir.MemoryLocationSet):
            continue
        name = alloc.memorylocations[0].name
        if alloc.kind == "ExternalInput":
            if name != pname:
                in_names.append(name)
        elif alloc.kind == "ExternalOutput":
            out_names.append(name)
            out_avals.append(
                jax.core.ShapedArray(tuple(alloc.tensor_shape), mybir.dt.np(alloc.dtype))
            )
    assert set(in_names) == {"xt", "wt", "be"}, in_names
    assert out_names == ["acts"], out_names
    all_in_names = in_names + out_names + ([pname] if pname else [])

    def _body(*args):
        operands = list(args)
        if pname:
            operands.append(partition_id_tensor())
        outs = _bass_exec_p.bind(
            *operands,
            out_avals=tuple(out_avals),
            in_names=tuple(all_in_names),
            out_names=tuple(out_names),
            lowering_input_output_aliases=(),
            sim_require_finite=True,
            sim_require_nnan=True,
            nc=nc,
        )
        return tuple(outs)

    devices = jax.devices()[:NCORES]
    assert len(devices) == NCORES, f"need {NCORES} neuron cores, got {len(devices)}"
    mesh = Mesh(np.asarray(devices), ("core",))
    arg_names = in_names + out_names
    in_specs = tuple(
        PartitionSpec() if nm == "xt" else PartitionSpec("core") for nm in arg_names
    )
    sharded = jax.jit(
        shard_map(
            _body,
            mesh=mesh,
            in_specs=in_specs,
            out_specs=(PartitionSpec("core"),),
            check_rep=False,
        )
    )

    from jax.sharding import NamedSharding

    # device-resident zero output-init buffers, uploaded once and reused
    zeros = [
        jax.device_put(
            np.zeros((NCORES * a.shape[0], *a.shape[1:]), a.dtype),
            NamedSharding(mesh, PartitionSpec("core")),
        )
        for a in out_avals
    ]

    def runner(xt, wt_concat, be_concat):
        args = {"xt": xt, "wt": wt_concat, "be": be_concat}
        out = sharded(*[args[nm] for nm in in_names], *zeros)
        return np.asarray(out[0])  # [DD, B]

    _STATE["runner"] = runner
    return runner


def _prep_inputs(x, W_enc, b_enc, b_dec):
    xc = (x.astype(np.float32) - b_dec.astype(np.float32)[None, :]).astype(np.float32)
    xt = _pack_x(xc)
    wt_concat = np.concatenate(
        [
            _pack_w(np.ascontiguousarray(W_enc[c * FSH : (c + 1) * FSH], dtype=np.float32))
            for c in range(NCORES)
        ],
        axis=0,
    )
    be_concat = np.concatenate(
        [
            np.ascontiguousarray(
                b_enc[c * FSH : (c + 1) * FSH].astype(np.float32).reshape(FT, 128).T
            )
            for c in range(NCORES)
        ],
        axis=0,
    )
    return xt, wt_concat, be_concat


def _run_device(x, W_enc, b_enc, b_dec, trace=False, trace_kwargs=None):
    if trace:
        # profiling path via run_bass_kernel_spmd (NTFF capture)
        from concourse.bass_utils import run_bass_kernel_spmd

        nc = _get_nc()
        xc = (x.astype(np.float32) - b_dec.astype(np.float32)[None, :]).astype(
            np.float32
        )
        xt = _pack_x(xc)
        in_maps = []
        for c in range(NCORES):
            in_maps.append(
                {
                    "xt": xt,
                    "wt": _pack_w(
                        np.ascontiguousarray(
                            W_enc[c * FSH : (c + 1) * FSH], dtype=np.float32
                        )
                    ),
                    "be": np.ascontiguousarray(
                        b_enc[c * FSH : (c + 1) * FSH]
                        .astype(np.float32)
                        .reshape(FT, 128)
                        .T
                    ),
                }
            )
        res = run_bass_kernel_spmd(
            nc, in_maps, list(range(NCORES)), trace=True, **(trace_kwargs or {})
        )
        _STATE["last_result"] = res
        return np.concatenate(
            [res.results[c]["acts"] for c in range(NCORES)], axis=0
        )

    runner = _get_runner()
    xt, wt_concat, be_concat = _prep_inputs(x, W_enc, b_enc, b_dec)
    return runner(xt, wt_concat, be_concat)


def _exact_vals(x32, W32, be64, f_idx, b_idx):
    """Accurate fp32 recompute of pre-relu acts at (b, f) pairs.

    Grouped by batch row so each group is a single BLAS sgemv — same
    accuracy class as the reference's own fp32 einsum.
    """
    n = len(f_idx)
    if n == 0:
        return np.zeros(0, np.float64)
    order = np.argsort(b_idx, kind="stable")
    fs, bs = f_idx[order], b_idx[order]
    ub, starts = np.unique(bs, return_index=True)
    ends = np.append(starts[1:], n)
    out = np.empty(n, np.float32)
    for i, b in enumerate(ub):
        s, e = starts[i], ends[i]
        out[s:e] = W32[fs[s:e]] @ x32[b]
    res = np.empty(n, np.float64)
    res[order] = out.astype(np.float64)
    return res + be64[f_idx]


def _select_topk(actsT, kb, x32, W32, be64, sigma):
    """Exact top-kb selection (reference semantics) from device f16 acts.

    Returns (b_idx, f_idx, values[fp32]) of the selected elements.
    actsT: [DD, B] float16 device activations.
    """
    DDl, Bl = actsT.shape
    total = DDl * Bl
    empty = (np.zeros(0, np.int64), np.zeros(0, np.int64), np.zeros(0, np.float32))
    if kb <= 0:
        return empty
    kb = min(kb, total)

    # abs error bound of device f16 acts vs exact fp32: fp8 e4m3 quantization
    # of both GEMM operands gives err std ~0.038*sigma; 0.25*sigma is a
    # ~6.5-std bound (f16 storage quantization is negligible next to it)
    errtot = max(0.25 * sigma, 1e-7)

    # conservative screen: comfortably more candidates than kb
    cnt = 0
    for t_frac in (2.45, 2.0, 1.5, 1.0, 0.5, 0.0):
        t_lo = t_frac * sigma
        m = actsT > np.float16(t_lo)
        cnt = int(m.sum())
        if cnt >= kb + max(1024, kb // 16) or t_frac == 0.0:
            break

    f_idx, b_idx = np.nonzero(m)
    vals = actsT[m].astype(np.float32)

    if cnt <= kb:
        # everything positive is selected (selected zeros are no-ops)
        ex = _exact_vals(x32, W32, be64, f_idx, b_idx)
        keep = ex > 0
        return (
            b_idx[keep],
            f_idx[keep],
            np.maximum(ex[keep], 0.0).astype(np.float32),
        )

    part = np.partition(vals, cnt - kb)
    tau_dev = float(part[cnt - kb])

    band = 2.5 * errtot
    for _ in range(24):
        refine = vals > tau_dev - band
        nr = int(refine.sum())
        if nr < kb:
            band *= 2.0
            continue
        fr, br = f_idx[refine], b_idx[refine]
        ex = _exact_vals(x32, W32, be64, fr, br)
        flat = br.astype(np.int64) * DDl + fr.astype(np.int64)
        # reference order: value desc, flat index asc on ties
        order = np.lexsort((flat, -ex))
        take = order[:kb]
        tau_exact = float(ex[take[-1]])
        # excluded elements either have f16 <= tau_dev - band or were below
        # the screen threshold t_lo, so their exact value is at most
        # max(tau_dev - band, t_lo) + errtot; selection is airtight iff
        # tau_exact is above that.
        excl_hi = max(tau_dev - band, t_lo) + errtot
        if tau_exact > excl_hi or (band > 2.0 * sigma + 1.0):
            vsel = np.maximum(ex[take], 0.0).astype(np.float32)
            return (br[take], fr[take], vsel)
        band *= 2.0
        if (tau_dev - band < t_lo + errtot or tau_exact <= t_lo + errtot) and t_lo > 0:
            # widen past the screen: fall back to all-positives screen
            m = actsT > np.float16(0.0)
            cnt = int(m.sum())
            f_idx, b_idx = np.nonzero(m)
            vals = actsT[m].astype(np.float32)
            t_lo = 0.0
            if cnt <= kb:
                ex = _exact_vals(x32, W32, be64, f_idx, b_idx)
                keep = ex > 0
                return (
                    b_idx[keep],
                    f_idx[keep],
                    np.maximum(ex[keep], 0.0).astype(np.float32),
                )
            part = np.partition(vals, cnt - kb)
            tau_dev = float(part[cnt - kb])
    raise RuntimeError("top-k band search failed to converge")


def _kernel_numpy_fallback(x, W_enc, b_enc, b_dec, k):
    x32 = x.astype(np.float32)
    acts = np.maximum(
        (x32 - b_dec.astype(np.float32)) @ W_enc.astype(np.float32).T
        + b_enc.astype(np.float32),
        0.0,
    )
    flat = acts.reshape(-1)
    kb = int(k) * x.shape[0]
    if kb <= 0:
        return np.zeros_like(acts)
    kb = min(kb, flat.size)
    idx = np.argpartition(flat, flat.size - kb)[flat.size - kb :]
    # exact reference tie-break: value desc, index asc
    order = np.lexsort((idx, -flat[idx].astype(np.float64)))
    idx = idx[order[:kb]]
    out = np.zeros_like(flat)
    out[idx] = flat[idx]
    return out.reshape(acts.shape)


def kernel(x, W_enc, b_enc, b_dec, k):
    x = np.asarray(x)
    W_enc = np.asarray(W_enc)
    b_enc = np.asarray(b_enc)
    b_dec = np.asarray(b_dec)
    kb = int(k) * x.shape[0]

    if x.shape != (B, D) or W_enc.shape != (DD, D):
        return _kernel_numpy_fallback(x, W_enc, b_enc, b_dec, k)

    actsT = _run_device(x, W_enc, b_enc, b_dec)  # [DD, B] f16

    if not np.all(np.isfinite(actsT[:: max(1, DD // 256)])) or np.any(
        actsT[:: max(1, DD // 256)] == np.inf
    ):
        return _kernel_numpy_fallback(x, W_enc, b_enc, b_dec, k)

    x32 = (x.astype(np.float32) - b_dec.astype(np.float32)[None, :]).astype(np.float32)
    W32 = np.ascontiguousarray(W_enc.astype(np.float32))
    be64 = b_enc.astype(np.float64)

    sub = actsT[:: max(1, DD // 1024)].astype(np.float32)
    sigma = float(np.sqrt(2.0 * np.mean(np.square(sub))))
    if not np.isfinite(sigma) or sigma <= 0:
        sigma = 1.0

    b_sel, f_sel, v_sel = _select_topk(actsT, kb, x32, W32, be64, sigma)

    out = np.zeros((B, DD), np.float32)
    out[b_sel, f_sel] = v_sel
    return out
